# revision 38
# baseline (speedup 1.0000x reference)
"""Trainium2 Bass kernel for nn_BoundarySeg (segment_reduce).

out[b, j, 0:H]   = sum_{i>=j} A[b, j, i] * h[b, i, :]
out[b, j, H:2H]  = h[b, j, :] * sum_{i>=j} A[b, j, i]

Shapes: A [8, 2048, 2048] f32, h [8, 2048, 256] f32 -> out [8, 2048, 512] f32.
Sharding: data-parallel over batch; core c computes batch c.

Per-core design (L=2048 in 16 tiles of 128, H=256), HBM floor ~40.5us
(A upper panels 8.5MB + h 2MB reads + out 4MB writes @ 358GB/s):
  - A panels stream via SWDGE (gpsimd) DMA with an in-flight fp32->bf16
    cast, in chunks of <=8 tiles (512KB HBM read each). bf16 halves PE
    transpose cost (1 cyc/row), makes the PSUM->SBUF copies eligible for
    the DVE 2x mode, and enables FWL weight loads for the matmuls.
  - h loads via HWDGE on the scalar ring (2x1MB), cast to bf16 on DVE
    into [128(p), 16(t), 258]; cols 256/257 are ones so the masked
    row-sum falls out of the main matmul as an extra column.
  - Per j-tile jc: transpose each 128x128 block on PE through PSUM
    (bf16), copy PSUM->SBUF on DVE (first chunk fuses the i>=j diagonal
    mask via tensor_tensor) or ACT (second chunk).
  - acc[j, n] += At_k^T @ h_ext over i-tiles >= jc (bf16 MMs, fp32 PSUM
    accumulate, N=258). first half = acc[:, 0:256]; second half =
    h[jc] * acc[:, 256] (per-partition scale).
  - Output chains (rowsum copy, first-half copy, second-half scale,
    store) alternate between an ACT flavor and a DVE flavor so
    consecutive panels drain in parallel; a chain is emitted CHAIN_LAG
    panels after its matmuls, which guarantees the matmuls have retired
    and the chain never head-of-line blocks the panel-copy stream.
  - Matmuls run DEPTH panels behind the panel loads so the PE stream
    never blocks the next panel's transposes; 16 panel buffers mean the
    A stream never waits on buffer recycling.
"""

import os
import sys

import numpy as np

sys.path.insert(0, "/opt/trn_rl_repo")

import concourse.bass as bass  # noqa: E402
import concourse.bacc as bacc  # noqa: E402
import concourse.tile as tile  # noqa: E402
from concourse import mybir  # noqa: E402
from concourse.bass_utils import run_bass_kernel_spmd  # noqa: E402
from concourse.masks import make_identity, make_lower_triangular  # noqa: E402

B, L, H = 8, 2048, 256
P = 128
CH = 8  # tiles per PSUM transpose group
DEPTH = 3  # panels the matmul stream runs behind the loads
CHAIN_LAG = 1  # extra panels the output chain runs behind the matmuls

DT = mybir.dt.float32
BF = mybir.dt.bfloat16

# Results of the last run (exec_time_ns etc.) for the test harness.
LAST_RESULTS = None
_NC_CACHE = {}


def _build_nc(L=L, H=H):
    NT = L // P
    HE = H + 2  # even N; col H = ones (rowsum), col H+1 unused

    nc = bacc.Bacc(None, target_bir_lowering=False)
    a_dram = nc.dram_tensor("a", [L, L], DT, kind="ExternalInput")
    h_dram = nc.dram_tensor("h", [L, H], DT, kind="ExternalInput")
    out_dram = nc.dram_tensor("out", [L, 2 * H], DT, kind="ExternalOutput")

    with tile.TileContext(nc) as tc:
        with (
            tc.tile_pool(name="const", bufs=1) as const_pool,
            tc.tile_pool(name="hpool", bufs=1) as h_pool,
            tc.tile_pool(name="apanel", bufs=16) as a_pool,
            tc.tile_pool(name="atT", bufs=5) as at_pool,
            tc.tile_pool(name="tp", bufs=3, space=bass.MemorySpace.PSUM) as tp_pool,
            tc.tile_pool(name="acc", bufs=5, space=bass.MemorySpace.PSUM) as acc_pool,
            tc.tile_pool(name="outsb", bufs=6) as out_pool,
            tc.tile_pool(name="small", bufs=4) as small_pool,
        ):
            # h: every matmul depends on it, so it must land ASAP — quarters
            # alternate across both HWDGE rings (the SDMA engines round-robin
            # per queue, so two queues get ~2/3 of the engine pool while the
            # A-panel stream warms up). The bf16 casts split across DVE and
            # ACT so neither engine's panel-copy stream queues behind a cast
            # that is still waiting for its h quarter to land.
            h_stage = h_pool.tile([P, NT, H], DT)
            h_all = h_pool.tile([P, NT, HE], BF)
            h_re = h_dram[:].rearrange("(t p) n -> p t n", p=P)
            nc.vector.memset(h_all[:, :, H:HE], 1.0)
            q = NT // 4
            for i in range(4):
                ring = nc.sync if i % 2 == 0 else nc.scalar
                ring.dma_start(
                    out=h_stage[:, i * q : (i + 1) * q, :],
                    in_=h_re[:, i * q : (i + 1) * q, :],
                )
            for i in range(4):
                if i < 2:
                    nc.vector.tensor_copy(
                        h_all[:, i * q : (i + 1) * q, 0:H],
                        h_stage[:, i * q : (i + 1) * q, :],
                    )
                else:
                    nc.scalar.copy(
                        h_all[:, i * q : (i + 1) * q, 0:H],
                        h_stage[:, i * q : (i + 1) * q, :],
                    )

            # Constants: identity for PE transposes; cmask for the diagonal
            # block of each panel ([i(part), j(free)], keep i >= j), with
            # ones past column P so one tensor_tensor covers a whole chunk.
            identity = const_pool.tile([P, P], BF)
            make_identity(nc, identity[:])
            mask_src = const_pool.tile([P, P], BF)
            make_lower_triangular(nc, mask_src[:], val=1.0, diag=True)
            cmask = const_pool.tile([P, CH * P], BF)
            nc.vector.tensor_copy(cmask[:, 0:P], mask_src[:])
            nc.vector.memset(cmask[:, P : CH * P], 1.0)

            # Warmup transpose: absorbs the gpsimd->PE wait for `identity`.
            wtp = tp_pool.tile([P, CH * P], BF, tag="tp")
            nc.tensor.transpose(wtp[:, 0:P], identity[:], identity[:])

            def emit_matmuls(jc, atT):
                ntiles = NT - jc
                acc = acc_pool.tile([P, HE], DT, tag="acc")
                for k in range(ntiles):
                    nc.tensor.matmul(
                        acc[:],
                        atT[:, k * P : (k + 1) * P],
                        h_all[:, jc + k, :],
                        start=(k == 0),
                        stop=(k == ntiles - 1),
                    )
                return acc

            def emit_chain(jc, acc, split=False):
                # Chains alternate ACT/DVE so consecutive panels drain in
                # parallel. A chain for panel jc is only emitted after panel
                # jc+DEPTH+CHAIN_LAG's copies, which guarantees (by PE FIFO
                # order) that MM(jc) has already finished — so a chain never
                # head-of-line blocks the copy stream on its engine.
                # `split` (used for the final flush, when the copy streams
                # are done and HOL-blocking is impossible) puts the first-
                # half copy on the opposite engine so a single chain's
                # critical path is rowsum+out2 only.
                out_sb = out_pool.tile([P, 2 * H], DT, tag="outsb")
                rowsum = small_pool.tile([P, 1], DT, tag="rowsum")
                act_flavor = jc % 2 == 0
                if act_flavor:
                    nc.scalar.copy(rowsum[:], acc[:, H : H + 1])
                    if split:
                        nc.vector.tensor_copy(out_sb[:, 0:H], acc[:, 0:H])
                    else:
                        nc.scalar.copy(out_sb[:, 0:H], acc[:, 0:H])
                    nc.scalar.activation(
                        out_sb[:, H : 2 * H],
                        h_all[:, jc, 0:H],
                        mybir.ActivationFunctionType.Copy,
                        scale=rowsum[:],
                    )
                    nc.scalar.dma_start(out_dram[jc * P : (jc + 1) * P, :], out_sb[:])
                else:
                    nc.vector.tensor_copy(rowsum[:], acc[:, H : H + 1])
                    if split:
                        nc.scalar.copy(out_sb[:, 0:H], acc[:, 0:H])
                    else:
                        nc.vector.tensor_copy(out_sb[:, 0:H], acc[:, 0:H])
                    nc.vector.tensor_scalar(
                        out_sb[:, H : 2 * H],
                        h_all[:, jc, 0:H],
                        rowsum[:],
                        None,
                        mybir.AluOpType.mult,
                    )
                    nc.sync.dma_start(out_dram[jc * P : (jc + 1) * P, :], out_sb[:])

            pending = []  # (jc, atT) whose matmuls run DEPTH panels later
            chain_pending = []  # (jc, acc) whose output chain runs later still

            def pop_pending(target_depth, chain_target=CHAIN_LAG):
                while len(pending) > target_depth:
                    jc0, atT0 = pending.pop(0)
                    chain_pending.append((jc0, emit_matmuls(jc0, atT0)))
                    while len(chain_pending) > chain_target:
                        emit_chain(*chain_pending.pop(0))

            for jc in range(NT):
                ntiles = NT - jc
                # One SWDGE cast-DMA per panel: up to 8KB contiguous per
                # partition on the HBM side, fewer triggers and semaphores.
                a_chunk = a_pool.tile([P, NT * P], BF, tag="apanel")
                nc.gpsimd.dma_start(
                    a_chunk[:, 0 : ntiles * P],
                    a_dram[jc * P : (jc + 1) * P, jc * P : L],
                )
                atT = at_pool.tile([P, ntiles * P], BF, tag="atT")
                for g0 in range(0, ntiles, CH):
                    gn = min(CH, ntiles - g0)
                    tp = tp_pool.tile([P, CH * P], BF, tag="tp")
                    for k in range(gn):
                        nc.tensor.transpose(
                            tp[:, k * P : (k + 1) * P],
                            a_chunk[:, (g0 + k) * P : (g0 + k + 1) * P],
                            identity[:],
                        )
                    if g0 == 0:
                        nc.vector.tensor_tensor(
                            atT[:, 0 : gn * P],
                            tp[:, 0 : gn * P],
                            cmask[:, 0 : gn * P],
                            mybir.AluOpType.mult,
                        )
                    else:
                        nc.scalar.copy(
                            atT[:, g0 * P : (g0 + gn) * P], tp[:, 0 : gn * P]
                        )

                pending.append((jc, atT))
                # Shrink both lags positionally in the endgame: matmul
                # batches whose panels have long arrived must be emitted
                # before the final transposes in the PE FIFO, and their
                # chains before the final copies in the DVE/ACT FIFOs —
                # otherwise ready work queues behind ops that wait on the
                # very last panel's DMA.
                d = NT - 1 - jc
                pop_pending(min(DEPTH, d), min(CHAIN_LAG, max(0, d - 1)))

            # Emit the remaining matmuls but keep their chains for the split
            # flush (both copy streams are done by then, so splitting each
            # chain's first-half copy onto the opposite engine is HOL-safe
            # and shortens the per-panel drain latency).
            pop_pending(0, len(pending) + len(chain_pending) + 1)
            for jc0, acc0 in chain_pending:
                emit_chain(jc0, acc0, split=True)

    nc.finalize()
    return nc


def kernel(span_adjacency, bound_hidden):
    global LAST_RESULTS
    a = np.ascontiguousarray(np.asarray(span_adjacency, dtype=np.float32))
    h = np.ascontiguousarray(np.asarray(bound_hidden, dtype=np.float32))
    assert a.shape == (B, L, L) and h.shape == (B, L, H), (a.shape, h.shape)

    key = "full"
    if key not in _NC_CACHE:
        _NC_CACHE[key] = _build_nc()
    nc = _NC_CACHE[key]

    in_maps = [{"a": a[b], "h": h[b]} for b in range(B)]
    res = run_bass_kernel_spmd(
        nc,
        in_maps,
        core_ids=list(range(B)),
        trace=bool(os.environ.get("KERNEL_TRACE")),
    )
    LAST_RESULTS = res
    out = np.stack([res.results[b]["out"] for b in range(B)], axis=0)
    return out
